# revision 25
# baseline (speedup 1.0000x reference)
"""Bass/Trainium2 kernel for nn_ContrastiveLoss_18502719111626.

Reference math:
    mask_i = (sum_d latent[i,d] != 0)
    ln     = latent / max(||latent_i||, 1e-8)
    total  = einsum('i,ij,j->', mask, ln @ ln.T, mask) - sum(mask)
    out    = 0.01 * total / (2 * N)

Key identity: einsum('i,ij,j->', m, ln@ln.T, m) == ||sum_i m_i * ln_i||^2,
so the N x N similarity matrix is never needed. Each core streams its
1024-row shard once (memory-roofline) and reduces it to per-partition
partials [128, 65] = [weighted column sums | mask count]; the host
finishes the partition/core sum (the same gather step that merges the 8
cores) and computes total = ||s||^2 - cnt.

Raw bacc (no TileContext): hand-rolled semaphores cost one light
all-engine barrier at block end instead of TileContext's
drain + barrier + semaphore-range-clear + barrier epilogue, and the
output DMA needs no in-program completion wait (the NEFF teardown's
queue drain already orders it before the host reads outputs).

Per-core dataflow (shard [1024, 64] f32), layout [128, 512] where
partition p holds shard rows 8p..8p+7 (one contiguous 2KB DRAM line per
partition -> 128 big DMA descriptors instead of 512 small ones). The
load is split in column halves so compute on half A overlaps the
in-flight half B:
    X[128, 8, 64]  (r = row-within-partition, d = feature)
    sq_h  = X_h^2                     (ScalarE Square, one per half)
    rs4_h = sum_d X_h                 (VectorE reduce [128,4,64]->[128,4])
    ss4_h = sum_d sq_h                (VectorE reduce)
    norm  = sqrt(ss8 + eps^2)         (ScalarE, bias tile folds eps clamp)
    cnt_p = sum_r (rs8 != 0)          (VectorE tensor_scalar accum -> wse[:,64])
    scale8 = (rs8 != 0) * (1/norm)    (VectorE reciprocal + scalar_tensor_tensor)
    w     = X * bcast(scale8)         (VectorE tensor_tensor, [128,512])
    wse[:, :64] = sum_r w             (VectorE strided reduce [128,64,8])
    partials[128, 65] -> DRAM
"""

import numpy as np

N = 8192
D = 64
NCORES = 8
ROWS = N // NCORES  # 1024 rows per core
R = ROWS // 128  # 8 rows per partition
H = R // 2  # rows per half
COF1 = 0.01
EPS = 1e-8

_prog = None


def _build():
    import concourse.bacc as bacc
    import concourse.mybir as mybir

    f32 = mybir.dt.float32
    AF = mybir.ActivationFunctionType
    ALU = mybir.AluOpType
    AX = mybir.AxisListType

    # Bacc (not plain Bass): its compile() runs generate_event_semaphores,
    # which splits multi-sem sync waits into EventSemaphore instructions --
    # walrus rejects >1 wait per instruction.
    # Same-engine RAW chains execute in order on HW (the tile-built
    # variants rely on this too); the raw-mode race detector would demand
    # a semaphore per edge, so it is off. Cross-engine edges are synced
    # explicitly below.
    nc = bacc.Bacc(None, detect_race_conditions=False, monotonic_sem_count=0)
    x_in = nc.declare_dram_parameter("latent", [ROWS, D], f32, isOutput=False)
    out_p = nc.declare_dram_parameter("partials", [128, D + 1], f32, isOutput=True)

    # Only the SP HWDGE ring is used; dropping the unused Act-HWDGE and
    # Pool-SWDGE ring declarations shrinks the NEFF's ring setup/teardown.
    nc.m.queues = [q for q in nc.m.queues if q.name == "qSPDynamicHW"]

    HD = H * D  # 256 columns per half
    xv = x_in.rearrange("(p r) d -> p (r d)", p=128)

    import contextlib

    with contextlib.ExitStack() as ctx:
        E = ctx.enter_context
        block = E(nc.Block(no_gpsimd_drain=True))
        s_eps = E(nc.semaphore("s_eps"))
        s_a = E(nc.semaphore("s_a"))
        s_b = E(nc.semaphore("s_b"))
        s_sq = E(nc.semaphore("s_sq"))
        s_rs = E(nc.semaphore("s_rs"))
        s_norm = E(nc.semaphore("s_norm"))
        s_v = E(nc.semaphore("s_v"))
        s_wse = E(nc.semaphore("s_wse"))
        s_out = E(nc.semaphore("s_out"))
        X = E(nc.sbuf_tensor("X", [128, R * D], f32))
        sq = E(nc.sbuf_tensor("sq", [128, R * D], f32))
        w = E(nc.sbuf_tensor("w", [128, R * D], f32))
        rs8 = E(nc.sbuf_tensor("rs8", [128, R], f32))
        ss8 = E(nc.sbuf_tensor("ss8", [128, R], f32))
        norm = E(nc.sbuf_tensor("norm", [128, R], f32))
        inv = E(nc.sbuf_tensor("inv", [128, R], f32))
        scale8 = E(nc.sbuf_tensor("scale8", [128, R], f32))
        masktmp = E(nc.sbuf_tensor("masktmp", [128, R], f32))
        wse = E(nc.sbuf_tensor("wse", [128, D + 1], f32))
        epsb = E(nc.sbuf_tensor("epsb", [128, 1], f32))
        warm = E(nc.sbuf_tensor("warm", [1, 1], f32))

        @block.sync
        def _(sync):
            sync.dma_start(X[:, :HD], xv[:, :HD]).then_inc(s_a, 16)
            sync.dma_start(X[:, HD:], xv[:, HD:]).then_inc(s_b, 16)
            # Output DMA: no in-program completion wait -- the NEFF
            # teardown drains the queues before outputs are read back.
            sync.wait_ge(s_wse, 2)
            sync.dma_start(out_p[:, :], wse[:, :]).then_inc(s_out, 16)

        @block.scalar
        def _(scalar):
            # Dummy sqrt as ScalarE's first activation: pulls in the
            # "sqrt_and_others" table (which also contains square), so
            # only one ACT_TABLE_LOAD happens instead of a second 1.3us
            # load right before the real sqrt.
            scalar.wait_ge(s_eps, 1)
            scalar.sqrt(warm[:, :], epsb[0:1, :])
            scalar.wait_ge(s_a, 16)
            scalar.activation(
                out=sq[:, :HD], in_=X[:, :HD], func=AF.Square
            ).then_inc(s_sq, 1)
            scalar.wait_ge(s_b, 16)
            scalar.activation(
                out=sq[:, HD:], in_=X[:, HD:], func=AF.Square
            ).then_inc(s_sq, 2)
            # norm = sqrt(ss + eps^2) == max(sqrt(ss), eps) up to fp32
            # rounding for any non-degenerate row.
            scalar.wait_ge(s_norm, 2)
            scalar.activation(
                out=norm[:, :], in_=ss8[:, :], func=AF.Sqrt, bias=epsb[:, :]
            ).then_inc(s_norm, 2)

        @block.vector
        def _(vector):
            vector.memset(epsb[:, :], EPS * EPS).then_inc(s_eps, 1)
            vector.wait_ge(s_a, 16)
            vector.tensor_reduce(
                out=rs8[:, 0:H],
                in_=X[:, :HD].rearrange("p (r d) -> p r d", r=H),
                axis=AX.X,
                op=ALU.add,
            ).then_inc(s_rs, 1)
            vector.wait_ge(s_sq, 1)
            vector.tensor_reduce(
                out=ss8[:, 0:H],
                in_=sq[:, :HD].rearrange("p (r d) -> p r d", r=H),
                axis=AX.X,
                op=ALU.add,
            ).then_inc(s_norm, 1)
            vector.wait_ge(s_b, 16)
            vector.tensor_reduce(
                out=rs8[:, H:R],
                in_=X[:, HD:].rearrange("p (r d) -> p r d", r=H),
                axis=AX.X,
                op=ALU.add,
            ).then_inc(s_rs, 1)
            vector.wait_ge(s_sq, 3)
            vector.tensor_reduce(
                out=ss8[:, H:R],
                in_=sq[:, HD:].rearrange("p (r d) -> p r d", r=H),
                axis=AX.X,
                op=ALU.add,
            ).then_inc(s_norm, 1)
            # cnt_p -> wse[:, 64] while ScalarE runs the sqrt. DVE
            # execution pipelines across sub-units, so every same-engine
            # RAW edge gets an explicit completion semaphore.
            vector.wait_ge(s_rs, 2)
            vector.tensor_scalar(
                masktmp[:, :],
                rs8[:, :],
                0.0, 0.0,
                op0=ALU.not_equal, op1=ALU.add,
                accum_out=wse[:, D : D + 1],
            ).then_inc(s_wse, 1)
            vector.wait_ge(s_norm, 4)
            vector.reciprocal(inv[:, :], norm[:, :]).then_inc(s_v, 1)
            vector.wait_ge(s_v, 1)
            vector.scalar_tensor_tensor(
                out=scale8[:, :],
                in0=rs8[:, :],
                scalar=0.0,
                in1=inv[:, :],
                op0=ALU.not_equal,
                op1=ALU.mult,
            ).then_inc(s_v, 1)
            vector.wait_ge(s_v, 2)
            vector.tensor_tensor(
                out=w[:, :].rearrange("p (r d) -> p r d", r=R),
                in0=X[:, :].rearrange("p (r d) -> p r d", r=R),
                in1=scale8[:, :].to_broadcast([128, R, D]),
                op=ALU.mult,
            ).then_inc(s_v, 1)
            vector.wait_ge(s_v, 3)
            vector.tensor_reduce(
                out=wse[:, :D],
                in_=w[:, :].rearrange("p (r d) -> p d r", r=R),
                axis=AX.X,
                op=ALU.add,
            ).then_inc(s_wse, 1)

        @block.tensor
        def _(tensor):
            pass

        @block.gpsimd
        def _(gpsimd):
            pass

    nc.compile()
    return nc


def _run_spmd(latent, trace=False, **kw):
    from concourse.bass_utils import run_bass_kernel_spmd

    global _prog
    if _prog is None:
        _prog = _build()
    in_maps = [
        {"latent": np.ascontiguousarray(latent[c * ROWS : (c + 1) * ROWS])}
        for c in range(NCORES)
    ]
    return run_bass_kernel_spmd(_prog, in_maps, list(range(NCORES)), trace=trace, **kw)


def _combine(results):
    parts = np.stack([results[c]["partials"] for c in range(NCORES)])  # [8, 128, 65]
    s = parts[:, :, :D].astype(np.float64).sum(axis=(0, 1))
    cnt = parts[:, :, D].astype(np.float64).sum()
    total = float(s @ s - cnt)
    return np.asarray(COF1 * total / (2.0 * N), dtype=np.float32)


def kernel(latent):
    latent = np.asarray(latent, dtype=np.float32)
    assert latent.shape == (N, D)
    return _combine(_run_spmd(latent).results)


# revision 26
# speedup vs baseline: 1.1878x; 1.1878x over previous
"""Bass/Trainium2 kernel for nn_ContrastiveLoss_18502719111626.

Reference math:
    mask_i = (sum_d latent[i,d] != 0)
    ln     = latent / max(||latent_i||, 1e-8)
    total  = einsum('i,ij,j->', mask, ln @ ln.T, mask) - sum(mask)
    out    = 0.01 * total / (2 * N)

Key identity: einsum('i,ij,j->', m, ln@ln.T, m) == ||sum_i m_i * ln_i||^2,
so the N x N similarity matrix is never needed. Each core streams its
1024-row shard once (memory-roofline) and reduces it to per-partition
partials [128, 65] = [weighted column sums | mask count]; the host
finishes the partition/core sum (the same gather step that merges the 8
cores) and computes total = ||s||^2 - cnt.

Raw bacc (no TileContext): hand-rolled semaphores cost one light
all-engine barrier at block end instead of TileContext's
drain + barrier + semaphore-range-clear + barrier epilogue, and the
output DMA needs no in-program completion wait (the NEFF teardown's
queue drain already orders it before the host reads outputs).

Per-core dataflow (shard [1024, 64] f32), layout [128, 512] where
partition p holds shard rows 8p..8p+7 (one contiguous 2KB DRAM line per
partition -> 128 big DMA descriptors instead of 512 small ones). The
load is split in column halves so compute on half A overlaps the
in-flight half B:
    X[128, 8, 64]  (r = row-within-partition, d = feature)
    sq_h  = X_h^2                     (ScalarE Square, one per half)
    rs4_h = sum_d X_h                 (VectorE reduce [128,4,64]->[128,4])
    ss4_h = sum_d sq_h                (VectorE reduce)
    norm  = sqrt(ss8 + eps^2)         (ScalarE, bias tile folds eps clamp)
    cnt_p = sum_r (rs8 != 0)          (VectorE tensor_scalar accum -> wse[:,64])
    scale8 = (rs8 != 0) * (1/norm)    (VectorE reciprocal + scalar_tensor_tensor)
    w     = X * bcast(scale8)         (VectorE tensor_tensor, [128,512])
    wse[:, :64] = sum_r w             (VectorE strided reduce [128,64,8])
    partials[128, 65] -> DRAM
"""

import numpy as np

N = 8192
D = 64
NCORES = 8
ROWS = N // NCORES  # 1024 rows per core
R = ROWS // 128  # 8 rows per partition
H = R // 2  # rows per half
COF1 = 0.01
EPS = 1e-8

_prog = None


def _build():
    import concourse.bacc as bacc
    import concourse.mybir as mybir

    f32 = mybir.dt.float32
    AF = mybir.ActivationFunctionType
    ALU = mybir.AluOpType
    AX = mybir.AxisListType

    # Bacc (not plain Bass): its compile() runs generate_event_semaphores,
    # which splits multi-sem sync waits into EventSemaphore instructions --
    # walrus rejects >1 wait per instruction.
    # Same-engine RAW chains execute in order on HW (the tile-built
    # variants rely on this too); the raw-mode race detector would demand
    # a semaphore per edge, so it is off. Cross-engine edges are synced
    # explicitly below.
    nc = bacc.Bacc(None, detect_race_conditions=False, monotonic_sem_count=0)
    x_in = nc.declare_dram_parameter("latent", [ROWS, D], f32, isOutput=False)
    out_p = nc.declare_dram_parameter("partials", [128, D + 1], f32, isOutput=True)

    # Only the SP HWDGE ring is used; dropping the unused Act-HWDGE and
    # Pool-SWDGE ring declarations shrinks the NEFF's ring setup/teardown.
    nc.m.queues = [q for q in nc.m.queues if q.name == "qSPDynamicHW"]

    HD = H * D  # 256 columns per half
    xv = x_in.rearrange("(p r) d -> p (r d)", p=128)

    import contextlib

    with contextlib.ExitStack() as ctx:
        E = ctx.enter_context
        block = E(nc.Block(no_gpsimd_drain=True))
        s_eps = E(nc.semaphore("s_eps"))
        s_a = E(nc.semaphore("s_a"))
        s_b = E(nc.semaphore("s_b"))
        s_sq = E(nc.semaphore("s_sq"))
        s_rs = E(nc.semaphore("s_rs"))
        s_norm = E(nc.semaphore("s_norm"))
        s_v = E(nc.semaphore("s_v"))
        s_wse = E(nc.semaphore("s_wse"))
        s_out = E(nc.semaphore("s_out"))
        X = E(nc.sbuf_tensor("X", [128, R * D], f32))
        sq = E(nc.sbuf_tensor("sq", [128, R * D], f32))
        w = E(nc.sbuf_tensor("w", [128, R * D], f32))
        rs8 = E(nc.sbuf_tensor("rs8", [128, R], f32))
        ss8 = E(nc.sbuf_tensor("ss8", [128, R], f32))
        norm = E(nc.sbuf_tensor("norm", [128, R], f32))
        inv = E(nc.sbuf_tensor("inv", [128, R], f32))
        scale8 = E(nc.sbuf_tensor("scale8", [128, R], f32))
        masktmp = E(nc.sbuf_tensor("masktmp", [128, R], f32))
        wse = E(nc.sbuf_tensor("wse", [128, D + 1], f32))
        epsb = E(nc.sbuf_tensor("epsb", [128, 1], f32))
        warm = E(nc.sbuf_tensor("warm", [1, 1], f32))

        @block.sync
        def _(sync):
            sync.dma_start(X[:, :HD], xv[:, :HD]).then_inc(s_a, 16)
            sync.dma_start(X[:, HD:], xv[:, HD:]).then_inc(s_b, 16)
            # Output DMA: no in-program completion wait -- the NEFF
            # teardown drains the queues before outputs are read back.
            sync.wait_ge(s_wse, 2)
            sync.dma_start(out_p[:, :], wse[:, :]).then_inc(s_out, 16)

        @block.scalar
        def _(scalar):
            # Dummy sqrt as ScalarE's first activation: pulls in the
            # "sqrt_and_others" table (which also contains square), so
            # only one ACT_TABLE_LOAD happens instead of a second 1.3us
            # load right before the real sqrt.
            scalar.wait_ge(s_eps, 1)
            scalar.activation(
                out=warm[:, :], in_=epsb[0:1, :], func=AF.Sqrt,
                bias=epsb[0:1, :],
            )
            # bias=eps^2 on the Squares is an fp32 no-op ((x+1e-16)^2
            # rounds to x^2) but keeps every activation bias an SBUF AP,
            # so no const-AP pool is referenced and the four const
            # memsets in the preamble become dead (deleted below) -- the
            # entry barrier stops waiting on them.
            scalar.wait_ge(s_a, 16)
            scalar.activation(
                out=sq[:, :HD], in_=X[:, :HD], func=AF.Square,
                bias=epsb[:, :],
            ).then_inc(s_sq, 1)
            scalar.wait_ge(s_b, 16)
            scalar.activation(
                out=sq[:, HD:], in_=X[:, HD:], func=AF.Square,
                bias=epsb[:, :],
            ).then_inc(s_sq, 2)
            # norm = sqrt(ss + eps^2) == max(sqrt(ss), eps) up to fp32
            # rounding for any non-degenerate row.
            scalar.wait_ge(s_norm, 2)
            scalar.activation(
                out=norm[:, :], in_=ss8[:, :], func=AF.Sqrt, bias=epsb[:, :]
            ).then_inc(s_norm, 2)

        @block.vector
        def _(vector):
            vector.memset(epsb[:, :], EPS * EPS).then_inc(s_eps, 1)
            vector.wait_ge(s_a, 16)
            vector.tensor_reduce(
                out=rs8[:, 0:H],
                in_=X[:, :HD].rearrange("p (r d) -> p r d", r=H),
                axis=AX.X,
                op=ALU.add,
            ).then_inc(s_rs, 1)
            vector.wait_ge(s_sq, 1)
            vector.tensor_reduce(
                out=ss8[:, 0:H],
                in_=sq[:, :HD].rearrange("p (r d) -> p r d", r=H),
                axis=AX.X,
                op=ALU.add,
            ).then_inc(s_norm, 1)
            vector.wait_ge(s_b, 16)
            vector.tensor_reduce(
                out=rs8[:, H:R],
                in_=X[:, HD:].rearrange("p (r d) -> p r d", r=H),
                axis=AX.X,
                op=ALU.add,
            ).then_inc(s_rs, 1)
            vector.wait_ge(s_sq, 3)
            vector.tensor_reduce(
                out=ss8[:, H:R],
                in_=sq[:, HD:].rearrange("p (r d) -> p r d", r=H),
                axis=AX.X,
                op=ALU.add,
            ).then_inc(s_norm, 1)
            # cnt_p -> wse[:, 64] while ScalarE runs the sqrt. DVE
            # execution pipelines across sub-units, so every same-engine
            # RAW edge gets an explicit completion semaphore.
            vector.wait_ge(s_rs, 2)
            vector.tensor_scalar(
                masktmp[:, :],
                rs8[:, :],
                0.0, 0.0,
                op0=ALU.not_equal, op1=ALU.add,
                accum_out=wse[:, D : D + 1],
            ).then_inc(s_wse, 1)
            vector.wait_ge(s_norm, 4)
            vector.reciprocal(inv[:, :], norm[:, :]).then_inc(s_v, 1)
            vector.wait_ge(s_v, 1)
            vector.scalar_tensor_tensor(
                out=scale8[:, :],
                in0=rs8[:, :],
                scalar=0.0,
                in1=inv[:, :],
                op0=ALU.not_equal,
                op1=ALU.mult,
            ).then_inc(s_v, 1)
            vector.wait_ge(s_v, 2)
            vector.tensor_tensor(
                out=w[:, :].rearrange("p (r d) -> p r d", r=R),
                in0=X[:, :].rearrange("p (r d) -> p r d", r=R),
                in1=scale8[:, :].to_broadcast([128, R, D]),
                op=ALU.mult,
            ).then_inc(s_v, 1)
            vector.wait_ge(s_v, 3)
            vector.tensor_reduce(
                out=wse[:, :D],
                in_=w[:, :].rearrange("p (r d) -> p d r", r=R),
                axis=AX.X,
                op=ALU.add,
            ).then_inc(s_wse, 1)

        @block.tensor
        def _(tensor):
            pass

        @block.gpsimd
        def _(gpsimd):
            pass

    # Drop const-pool memsets whose targets nothing reads: they run on
    # Pool ahead of the entry barrier and delay every engine's start.
    read_refs = set()
    for b in nc.m.functions[0].blocks:
        for i in b.instructions:
            for a in getattr(i, "ins", []) or []:
                r = getattr(a, "memsetref", None)
                if r:
                    read_refs.add(str(r))
    for b in nc.m.functions[0].blocks:
        b.instructions = [
            i
            for i in b.instructions
            if not (
                type(i).__name__ == "InstMemset"
                and "const-" in str(getattr(i.outs[0], "memsetref", ""))
                and str(getattr(i.outs[0], "memsetref", "")) not in read_refs
            )
        ]

    nc.compile()
    return nc


def _run_spmd(latent, trace=False, **kw):
    from concourse.bass_utils import run_bass_kernel_spmd

    global _prog
    if _prog is None:
        _prog = _build()
    in_maps = [
        {"latent": np.ascontiguousarray(latent[c * ROWS : (c + 1) * ROWS])}
        for c in range(NCORES)
    ]
    return run_bass_kernel_spmd(_prog, in_maps, list(range(NCORES)), trace=trace, **kw)


def _combine(results):
    parts = np.stack([results[c]["partials"] for c in range(NCORES)])  # [8, 128, 65]
    s = parts[:, :, :D].astype(np.float64).sum(axis=(0, 1))
    cnt = parts[:, :, D].astype(np.float64).sum()
    total = float(s @ s - cnt)
    return np.asarray(COF1 * total / (2.0 * N), dtype=np.float32)


def kernel(latent):
    latent = np.asarray(latent, dtype=np.float32)
    assert latent.shape == (N, D)
    return _combine(_run_spmd(latent).results)


# revision 27
# speedup vs baseline: 1.2989x; 1.0935x over previous
"""Bass/Trainium2 kernel for nn_ContrastiveLoss_18502719111626.

Reference math:
    mask_i = (sum_d latent[i,d] != 0)
    ln     = latent / max(||latent_i||, 1e-8)
    total  = einsum('i,ij,j->', mask, ln @ ln.T, mask) - sum(mask)
    out    = 0.01 * total / (2 * N)

Key identity: einsum('i,ij,j->', m, ln@ln.T, m) == ||sum_i m_i * ln_i||^2,
so the N x N similarity matrix is never needed. Each core streams its
1024-row shard once (memory-roofline) and reduces it to per-partition
partials [128, 65] = [weighted column sums | mask count]; the host
finishes the partition/core sum (the same gather step that merges the 8
cores) and computes total = ||s||^2 - cnt.

Raw bacc (no TileContext): hand-rolled semaphores cost one light
all-engine barrier at block end instead of TileContext's
drain + barrier + semaphore-range-clear + barrier epilogue, and the
output DMA needs no in-program completion wait (the NEFF teardown's
queue drain already orders it before the host reads outputs).

Per-core dataflow (shard [1024, 64] f32), layout [128, 512] where
partition p holds shard rows 8p..8p+7 (one contiguous 2KB DRAM line per
partition -> 128 big DMA descriptors instead of 512 small ones). The
load is split in column halves so compute on half A overlaps the
in-flight half B:
    X[128, 8, 64]  (r = row-within-partition, d = feature)
    sq_h  = X_h^2                     (ScalarE Square, one per half)
    rs4_h = sum_d X_h                 (VectorE reduce [128,4,64]->[128,4])
    ss4_h = sum_d sq_h                (VectorE reduce)
    norm  = sqrt(ss8 + eps^2)         (ScalarE, bias tile folds eps clamp)
    cnt_p = sum_r (rs8 != 0)          (VectorE tensor_scalar accum -> wse[:,64])
    scale8 = (rs8 != 0) * (1/norm)    (VectorE reciprocal + scalar_tensor_tensor)
    w     = X * bcast(scale8)         (VectorE tensor_tensor, [128,512])
    wse[:, :64] = sum_r w             (VectorE strided reduce [128,64,8])
    partials[128, 65] -> DRAM
"""

import numpy as np

N = 8192
D = 64
NCORES = 8
ROWS = N // NCORES  # 1024 rows per core
R = ROWS // 128  # 8 rows per partition
H = R // 2  # rows per half
COF1 = 0.01
EPS = 1e-8

_prog = None


def _build():
    import concourse.bacc as bacc
    import concourse.mybir as mybir

    f32 = mybir.dt.float32
    AF = mybir.ActivationFunctionType
    ALU = mybir.AluOpType
    AX = mybir.AxisListType

    # Bacc (not plain Bass): its compile() runs generate_event_semaphores,
    # which splits multi-sem sync waits into EventSemaphore instructions --
    # walrus rejects >1 wait per instruction.
    # Same-engine RAW chains execute in order on HW (the tile-built
    # variants rely on this too); the raw-mode race detector would demand
    # a semaphore per edge, so it is off. Cross-engine edges are synced
    # explicitly below.
    nc = bacc.Bacc(None, detect_race_conditions=False, monotonic_sem_count=0)
    x_in = nc.declare_dram_parameter("latent", [ROWS, D], f32, isOutput=False)
    out_p = nc.declare_dram_parameter("partials", [128, D + 1], f32, isOutput=True)

    # Only the SP HWDGE ring is used; dropping the unused Act-HWDGE and
    # Pool-SWDGE ring declarations shrinks the NEFF's ring setup/teardown.
    nc.m.queues = [q for q in nc.m.queues if q.name == "qSPDynamicHW"]

    HD = H * D  # 256 columns per half
    xv = x_in.rearrange("(p r) d -> p (r d)", p=128)

    import contextlib

    with contextlib.ExitStack() as ctx:
        E = ctx.enter_context
        block = E(nc.Block(no_gpsimd_drain=True))
        s_eps = E(nc.semaphore("s_eps"))
        s_a = E(nc.semaphore("s_a"))
        s_b = E(nc.semaphore("s_b"))
        s_sq = E(nc.semaphore("s_sq"))
        s_rs = E(nc.semaphore("s_rs"))
        s_norm = E(nc.semaphore("s_norm"))
        s_v = E(nc.semaphore("s_v"))
        s_wse = E(nc.semaphore("s_wse"))
        s_out = E(nc.semaphore("s_out"))
        X = E(nc.sbuf_tensor("X", [128, R * D], f32))
        sq = E(nc.sbuf_tensor("sq", [128, R * D], f32))
        w = E(nc.sbuf_tensor("w", [128, R * D], f32))
        rs8 = E(nc.sbuf_tensor("rs8", [128, R], f32))
        ss8 = E(nc.sbuf_tensor("ss8", [128, R], f32))
        norm = E(nc.sbuf_tensor("norm", [128, R], f32))
        inv = E(nc.sbuf_tensor("inv", [128, R], f32))
        scale8 = E(nc.sbuf_tensor("scale8", [128, R], f32))
        masktmp = E(nc.sbuf_tensor("masktmp", [128, R], f32))
        wse = E(nc.sbuf_tensor("wse", [128, D + 1], f32))
        epsb = E(nc.sbuf_tensor("epsb", [128, 1], f32))
        warm = E(nc.sbuf_tensor("warm", [1, 1], f32))

        @block.sync
        def _(sync):
            sync.dma_start(X[:, :HD], xv[:, :HD]).then_inc(s_a, 16)
            sync.dma_start(X[:, HD:], xv[:, HD:]).then_inc(s_b, 16)
            # Output DMA: no in-program completion wait -- the NEFF
            # teardown drains the queues before outputs are read back.
            sync.wait_ge(s_wse, 2)
            sync.dma_start(out_p[:, :], wse[:, :]).then_inc(s_out, 16)

        @block.scalar
        def _(scalar):
            # Dummy sqrt as ScalarE's first activation: pulls in the
            # "sqrt_and_others" table (which also contains square), so
            # only one ACT_TABLE_LOAD happens instead of a second 1.3us
            # load right before the real sqrt.
            scalar.wait_ge(s_eps, 1)
            scalar.activation(
                out=warm[:, :], in_=epsb[0:1, :], func=AF.Sqrt,
                bias=epsb[0:1, :],
            )
            # bias=eps^2 on the Squares is an fp32 no-op ((x+1e-16)^2
            # rounds to x^2) but keeps every activation bias an SBUF AP,
            # so no const-AP pool is referenced and the four const
            # memsets in the preamble become dead (deleted below) -- the
            # entry barrier stops waiting on them.
            scalar.wait_ge(s_a, 16)
            scalar.activation(
                out=sq[:, :HD], in_=X[:, :HD], func=AF.Square,
                bias=epsb[:, :],
            ).then_inc(s_sq, 1)
            scalar.wait_ge(s_b, 16)
            scalar.activation(
                out=sq[:, HD:], in_=X[:, HD:], func=AF.Square,
                bias=epsb[:, :],
            ).then_inc(s_sq, 2)
            # norm = sqrt(ss + eps^2) == max(sqrt(ss), eps) up to fp32
            # rounding for any non-degenerate row.
            scalar.wait_ge(s_norm, 2)
            scalar.activation(
                out=norm[:, :], in_=ss8[:, :], func=AF.Sqrt, bias=epsb[:, :]
            ).then_inc(s_norm, 2)

        @block.vector
        def _(vector):
            vector.memset(epsb[:, :], EPS * EPS).then_inc(s_eps, 1)
            vector.wait_ge(s_a, 16)
            vector.tensor_reduce(
                out=rs8[:, 0:H],
                in_=X[:, :HD].rearrange("p (r d) -> p r d", r=H),
                axis=AX.X,
                op=ALU.add,
            ).then_inc(s_rs, 1)
            vector.wait_ge(s_sq, 1)
            vector.tensor_reduce(
                out=ss8[:, 0:H],
                in_=sq[:, :HD].rearrange("p (r d) -> p r d", r=H),
                axis=AX.X,
                op=ALU.add,
            ).then_inc(s_norm, 1)
            vector.wait_ge(s_b, 16)
            vector.tensor_reduce(
                out=rs8[:, H:R],
                in_=X[:, HD:].rearrange("p (r d) -> p r d", r=H),
                axis=AX.X,
                op=ALU.add,
            ).then_inc(s_rs, 1)
            vector.wait_ge(s_sq, 3)
            vector.tensor_reduce(
                out=ss8[:, H:R],
                in_=sq[:, HD:].rearrange("p (r d) -> p r d", r=H),
                axis=AX.X,
                op=ALU.add,
            ).then_inc(s_norm, 1)
            # cnt_p -> wse[:, 64] while ScalarE runs the sqrt. DVE
            # execution pipelines across sub-units, so every same-engine
            # RAW edge gets an explicit completion semaphore.
            vector.wait_ge(s_rs, 2)
            vector.tensor_scalar(
                masktmp[:, :],
                rs8[:, :],
                0.0, 0.0,
                op0=ALU.not_equal, op1=ALU.add,
                accum_out=wse[:, D : D + 1],
            ).then_inc(s_wse, 1)
            vector.wait_ge(s_norm, 4)
            vector.reciprocal(inv[:, :], norm[:, :]).then_inc(s_v, 1)
            vector.wait_ge(s_v, 1)
            vector.scalar_tensor_tensor(
                out=scale8[:, :],
                in0=rs8[:, :],
                scalar=0.0,
                in1=inv[:, :],
                op0=ALU.not_equal,
                op1=ALU.mult,
            ).then_inc(s_v, 1)
            vector.wait_ge(s_v, 2)
            vector.tensor_tensor(
                out=w[:, :].rearrange("p (r d) -> p r d", r=R),
                in0=X[:, :].rearrange("p (r d) -> p r d", r=R),
                in1=scale8[:, :].to_broadcast([128, R, D]),
                op=ALU.mult,
            ).then_inc(s_v, 1)
            vector.wait_ge(s_v, 3)
            vector.tensor_reduce(
                out=wse[:, :D],
                in_=w[:, :].rearrange("p (r d) -> p d r", r=R),
                axis=AX.X,
                op=ALU.add,
            ).then_inc(s_wse, 1)

        @block.tensor
        def _(tensor):
            pass

        @block.gpsimd
        def _(gpsimd):
            pass

    # Hoist the two input-DMA descriptor generations into the preamble
    # block, ahead of the entry barrier: they only write X and signal
    # semaphores that the preamble reset long before, so they can overlap
    # the barrier and the data lands ~0.7us earlier.
    blocks = {b.name: b for b in nc.m.functions[0].blocks}
    main = blocks["main"]
    sp_body = next(
        b for b in nc.m.functions[0].blocks
        if any(type(i).__name__ == "InstDMACopy" for i in b.instructions)
    )
    in_dmas = [i for i in sp_body.instructions
               if type(i).__name__ == "InstDMACopy"][:2]
    sp_body.instructions = [i for i in sp_body.instructions
                            if i not in in_dmas]
    drain_idx = next(
        k for k, i in enumerate(main.instructions)
        if type(i).__name__ == "InstDrain"
        and i.engine == mybir.EngineType.SP
    )
    main.instructions = (
        main.instructions[:drain_idx]
        + in_dmas
        + main.instructions[drain_idx:]
    )

    # Drop const-pool memsets whose targets nothing reads: they run on
    # Pool ahead of the entry barrier and delay every engine's start.
    read_refs = set()
    for b in nc.m.functions[0].blocks:
        for i in b.instructions:
            for a in getattr(i, "ins", []) or []:
                r = getattr(a, "memsetref", None)
                if r:
                    read_refs.add(str(r))
    for b in nc.m.functions[0].blocks:
        b.instructions = [
            i
            for i in b.instructions
            if not (
                type(i).__name__ == "InstMemset"
                and "const-" in str(getattr(i.outs[0], "memsetref", ""))
                and str(getattr(i.outs[0], "memsetref", "")) not in read_refs
            )
        ]

    nc.compile()
    return nc


def _run_spmd(latent, trace=False, **kw):
    from concourse.bass_utils import run_bass_kernel_spmd

    global _prog
    if _prog is None:
        _prog = _build()
    in_maps = [
        {"latent": np.ascontiguousarray(latent[c * ROWS : (c + 1) * ROWS])}
        for c in range(NCORES)
    ]
    return run_bass_kernel_spmd(_prog, in_maps, list(range(NCORES)), trace=trace, **kw)


def _combine(results):
    parts = np.stack([results[c]["partials"] for c in range(NCORES)])  # [8, 128, 65]
    s = parts[:, :, :D].astype(np.float64).sum(axis=(0, 1))
    cnt = parts[:, :, D].astype(np.float64).sum()
    total = float(s @ s - cnt)
    return np.asarray(COF1 * total / (2.0 * N), dtype=np.float32)


def kernel(latent):
    latent = np.asarray(latent, dtype=np.float32)
    assert latent.shape == (N, D)
    return _combine(_run_spmd(latent).results)
